# revision 3
# baseline (speedup 1.0000x reference)
"""7x7 'same' 2D convolution over [128, 512, 512] f32, data-parallel on 8 NeuronCores.

Banded-Toeplitz formulation with 32x32 TensorEngine array packing: the PE
array runs as 16 independent 32x32 tiles. Each output-row window w (26 rows,
stride 26; window 19 is an 18-row tail) maps to tile position
(32*(w%4), 32*c): moving data = 32 input rows (26 loaded from HBM + 6-row
halo built by SBUF->SBUF DMA), stationary = the [32, 26] Toeplitz band
T_v[r', m] = w[r'-m, v]. The 7 column taps (v) accumulate into PSUM via
shifted moving views. 20 windows/image over 5 PSUM banks; the tail bank's
column group rotates with the image index to balance tile occupancy.

Inputs are cast fp16 and pre-staged host-side into a 5-slab layout
(partition 32r+p', slab q, col) = padded row 104q + 26r + p' for p' < 26
(halo partitions 26..31 of each strip are never transferred). PSUM banks are
evacuated (VectorE/ScalarE alternating) into densely packed fp16 tiles
([104, 2048] main + [96, 1024] tail-pair) so stores move exactly the output
bytes with 2-4KB DMA descriptors; the host un-permutes. Loads + halo copies
own the sync HWDGE ring; stores alternate scalar HWDGE / gpsimd SWDGE.
"""

import numpy as np

B, H, W = 128, 512, 512
KS = 7
PAD = (KS - 1) // 2          # 3
HP = H + 2 * PAD             # 518
N_CORES = 8
PER_CORE = B // N_CORES      # 16
TS = 26                      # output rows per 32x32 window
KW = 32                      # input rows per window
NW = 20                      # windows per image
TAILM = H - 19 * TS          # 18
TAILK = TAILM + KS - 1       # 24
NSLAB = 5
XROWS = 520                  # padded rows addressed by the slab layout


def _build_program():
    import concourse.bass as bass
    import concourse.tile as tile
    from concourse import bacc, mybir

    f16 = mybir.dt.float16
    f32 = mybir.dt.float32

    nc = bacc.Bacc("TRN2", target_bir_lowering=False, debug=False,
                   num_devices=N_CORES)
    x_ext = nc.declare_dram_parameter("x", [PER_CORE, 128, NSLAB * HP], f16,
                                      isOutput=False)
    t_ext = nc.declare_dram_parameter("toep", [128, KS * TS], f16,
                                      isOutput=False)
    # oa[img] packs windows 0..15: row 26c+m, col 512r+j = out row
    # 104c + 26r + m, col j
    oa_ext = nc.declare_dram_parameter("oa", [PER_CORE, 104, 2048], f16,
                                       isOutput=True)
    # ob[ip] packs the tails of images 2ip, 2ip+1: row 26j+m,
    # col 512*par + j' = img 2ip+par, out row 416 + 26j + m
    ob_ext = nc.declare_dram_parameter("ob", [PER_CORE // 2, 96, 1024], f16,
                                       isOutput=True)

    with tile.TileContext(nc) as tc:
        with (
            tc.tile_pool(name="toep", bufs=1) as toep_pool,
            tc.tile_pool(name="x", bufs=3) as x_pool,
            tc.tile_pool(name="oa", bufs=3) as oa_pool,
            tc.tile_pool(name="ob", bufs=2) as ob_pool,
            tc.tile_pool(name="psum", bufs=8, space="PSUM") as psum_pool,
        ):
            toep_sb = toep_pool.tile([128, KS * TS], f16)
            nc.sync.dma_start(out=toep_sb[:], in_=t_ext[:])

            ec = 0          # evac round-robin counter
            tile_b = None
            for img in range(PER_CORE):
                xt = x_pool.tile([128, NSLAB * HP], f16, name="xt",
                                 tag="xt")
                for r in range(4):
                    nc.sync.dma_start(out=xt[32 * r:32 * r + TS, :],
                                      in_=x_ext[img, 32 * r:32 * r + TS, :])
                # halo: strip r rows 26..31 = strip r+1 rows 0..5
                for r in range(3):
                    nc.sync.dma_start(
                        out=xt[32 * r + TS:32 * r + KW, :],
                        in_=xt[32 * (r + 1):32 * (r + 1) + 6, :])
                # strip 3 halo = strip 0 rows 0..5 of the next slab
                nc.sync.dma_start(out=xt[96 + TS:128, 0:4 * HP],
                                  in_=xt[0:6, HP:NSLAB * HP])

                ps = [psum_pool.tile([128, 512], f32, name=f"ps{k}",
                                     tag="acc") for k in range(5)]
                for wi in range(NW):
                    r = wi % 4
                    q = wi // 4
                    if wi < 16:
                        c, b, M, K = q, r, TS, KW
                    else:
                        j = wi - 16
                        c, b = (j + img) % 4, 4
                        M, K = (TS, KW) if j < 3 else (TAILM, TAILK)
                    for v in range(KS):
                        nc.tensor.matmul(
                            ps[b][32 * c:32 * c + M, :],
                            toep_sb[32 * r:32 * r + K,
                                    TS * v:TS * v + M],
                            xt[32 * r:32 * r + K,
                               HP * q + v:HP * q + v + 512],
                            start=(v == 0),
                            stop=(v == KS - 1),
                            tile_position=(32 * r, 32 * c),
                        )

                tile_a = oa_pool.tile([128, 2048], f16, name="oa",
                                      tag="oa")
                if img % 2 == 0:
                    tile_b = ob_pool.tile([128, 1024], f16, name="ob",
                                          tag="ob")

                def evac(dst, src):
                    nonlocal ec
                    if ec % 2 == 0:
                        nc.vector.tensor_copy(dst, src)
                    else:
                        nc.scalar.copy(dst, src)
                    ec += 1

                for wi in range(16):
                    r, c = wi % 4, wi // 4
                    evac(tile_a[32 * c:32 * c + TS,
                                512 * r:512 * r + 512],
                         ps[r][32 * c:32 * c + TS, :])
                for j in range(4):
                    c = (j + img) % 4
                    M = TS if j < 3 else TAILM
                    evac(tile_b[32 * j:32 * j + M,
                                512 * (img % 2):512 * (img % 2) + 512],
                         ps[4][32 * c:32 * c + M, :])

                # dense-pack in the DMA: DRAM rows 26c+m <- SBUF parts 32c+m
                for c in range(4):
                    nc.scalar.dma_start(
                        out=oa_ext[img, 26 * c:26 * c + TS, :],
                        in_=tile_a[32 * c:32 * c + TS, :])
                if img % 2 == 1:
                    for j in range(4):
                        M = TS if j < 3 else TAILM
                        nc.gpsimd.dma_start(
                            out=ob_ext[img // 2, 26 * j:26 * j + M, :],
                            in_=tile_b[32 * j:32 * j + M, :])
    nc.finalize()
    return nc


def _host_prep(x, w):
    x = np.asarray(x, dtype=np.float32)
    w = np.asarray(w, dtype=np.float32)
    xpad = np.zeros((B, XROWS, HP), dtype=np.float16)
    xpad[:, PAD:PAD + H, PAD:PAD + W] = x
    p = np.arange(128)
    q = np.arange(NSLAB)
    ridx = (104 * q[None, :] + 26 * (p[:, None] // 32)
            + np.minimum(p[:, None] % 32, 25))
    xslab = np.ascontiguousarray(
        xpad[:, ridx, :].reshape(B, 128, NSLAB * HP))
    toep = np.zeros((128, KS * TS), dtype=np.float16)
    w16 = w.astype(np.float16)
    m = np.arange(TS)
    for st in range(4):
        for v in range(KS):
            for d in range(KS):
                toep[32 * st + m + d, TS * v + m] = w16[d, v]
    return xslab, toep


def _execute(x, w, **run_kwargs):
    from concourse.bass_utils import run_bass_kernel_spmd

    xslab, toep = _host_prep(x, w)
    nc = _build_program()
    in_maps = [
        {"x": xslab[c * PER_CORE:(c + 1) * PER_CORE], "toep": toep}
        for c in range(N_CORES)
    ]
    last_err = None
    for _attempt in range(3):
        try:
            res = run_bass_kernel_spmd(nc, in_maps,
                                       core_ids=list(range(N_CORES)),
                                       **run_kwargs)
            break
        except Exception as e:  # transient NRT execute flakes -> retry
            last_err = e
    else:
        raise last_err
    out = np.empty((B, H, W), dtype=np.float32)
    for c in range(N_CORES):
        sl = slice(c * PER_CORE, (c + 1) * PER_CORE)
        oa = np.asarray(res.results[c]["oa"], dtype=np.float32)
        ob = np.asarray(res.results[c]["ob"], dtype=np.float32)
        oa6 = oa.reshape(PER_CORE, 4, 26, 4, 512)
        out[sl, :416, :] = oa6.transpose(0, 1, 3, 2, 4).reshape(
            PER_CORE, 416, 512)
        ob4 = ob.reshape(PER_CORE // 2, 96, 2, 512)
        out[sl, 416:, :] = ob4.transpose(0, 2, 1, 3).reshape(
            PER_CORE, 96, 512)
    return out, res


def kernel(x, w):
    out, _ = _execute(x, w)
    return out


# revision 5
# speedup vs baseline: 2.7978x; 2.7978x over previous
"""7x7 'same' 2D convolution over [128, 512, 512] f32, data-parallel on 8 NeuronCores.

Banded-Toeplitz formulation with 32x32 TensorEngine array packing: the PE
array runs as 16 independent 32x32 tiles. Each output-row window w (26 rows,
stride 26; window 19 is an 18-row tail) maps to tile position
(32*(w%4), 32*c): moving data = 32 input rows (26 loaded from HBM + 6-row
halo built by SBUF->SBUF DMA), stationary = the [32, 26] Toeplitz band
T_v[r', m] = w[r'-m, v]. The 7 column taps (v) accumulate into PSUM via
shifted moving views. 20 windows/image over 5 PSUM banks; the tail bank's
column group rotates with the image index to balance tile occupancy.

Inputs are cast fp16 and pre-staged host-side into a 5-slab layout
(partition 32r+p', slab q, col) = padded row 104q + 26r + p' for p' < 26
(halo partitions 26..31 of each strip are never transferred). PSUM banks are
evacuated (VectorE/ScalarE alternating) into densely packed fp16 tiles
([104, 2048] main + [96, 1024] tail-pair) so stores move exactly the output
bytes with 2-4KB DMA descriptors; the host un-permutes. Loads + halo copies
own the sync HWDGE ring; stores alternate scalar HWDGE / gpsimd SWDGE.
"""

import numpy as np

B, H, W = 128, 512, 512
KS = 7
PAD = (KS - 1) // 2          # 3
HP = H + 2 * PAD             # 518
N_CORES = 8
PER_CORE = B // N_CORES      # 16
TS = 26                      # output rows per 32x32 window
KW = 32                      # input rows per window
NW = 20                      # windows per image
TAILM = H - 19 * TS          # 18
TAILK = TAILM + KS - 1       # 24
NSLAB = 5
XROWS = 520                  # padded rows addressed by the slab layout


def _build_program():
    import concourse.bass as bass
    import concourse.tile as tile
    from concourse import bacc, mybir

    f16 = mybir.dt.float16
    f32 = mybir.dt.float32

    nc = bacc.Bacc("TRN2", target_bir_lowering=False, debug=False,
                   num_devices=N_CORES)
    x_ext = nc.declare_dram_parameter("x", [PER_CORE, 128, NSLAB * HP], f16,
                                      isOutput=False)
    t_ext = nc.declare_dram_parameter("toep", [128, KS * TS], f16,
                                      isOutput=False)
    # oa[img] packs windows 0..15: row 26c+m, col 512r+j = out row
    # 104c + 26r + m, col j
    oa_ext = nc.declare_dram_parameter("oa", [PER_CORE, 104, 2048], f16,
                                       isOutput=True)
    # ob[ip] packs the tails of images 2ip, 2ip+1: row 26j+m,
    # col 512*par + j' = img 2ip+par, out row 416 + 26j + m
    ob_ext = nc.declare_dram_parameter("ob", [PER_CORE // 2, 96, 1024], f16,
                                       isOutput=True)

    with tile.TileContext(nc) as tc:
        with (
            tc.tile_pool(name="toep", bufs=1) as toep_pool,
            tc.tile_pool(name="x", bufs=3) as x_pool,
            tc.tile_pool(name="oa", bufs=3) as oa_pool,
            tc.tile_pool(name="ob", bufs=2) as ob_pool,
            tc.tile_pool(name="psum", bufs=8, space="PSUM") as psum_pool,
        ):
            toep_sb = toep_pool.tile([128, KS * TS], f16)
            nc.sync.dma_start(out=toep_sb[:], in_=t_ext[:])

            xts = {}

            def load_img(i):
                xt = x_pool.tile([128, NSLAB * HP], f16, name="xt",
                                 tag="xt")
                for r in range(4):
                    nc.sync.dma_start(out=xt[32 * r:32 * r + TS, :],
                                      in_=x_ext[i, 32 * r:32 * r + TS, :])
                xts[i] = xt

            ec = 0          # evac round-robin counter
            tile_b = None
            load_img(0)
            load_img(1)
            for img in range(PER_CORE):
                # prefetch ahead so halo copies (which wait on their
                # image's load) never stall later loads in the ring FIFO
                if img + 2 < PER_CORE:
                    load_img(img + 2)
                xt = xts.pop(img)
                # halo: strip r rows 26..31 = strip r+1 rows 0..5
                for r in range(3):
                    nc.sync.dma_start(
                        out=xt[32 * r + TS:32 * r + KW, :],
                        in_=xt[32 * (r + 1):32 * (r + 1) + 6, :])
                # strip 3 halo = strip 0 rows 0..5 of the next slab
                nc.sync.dma_start(out=xt[96 + TS:128, 0:4 * HP],
                                  in_=xt[0:6, HP:NSLAB * HP])

                ps = [psum_pool.tile([128, 512], f32, name=f"ps{k}",
                                     tag="acc") for k in range(5)]
                # v-major emission: consecutive matmuls hit different tile
                # positions so all 16 PE tiles stream concurrently
                for v in range(KS):
                    for wi in range(NW):
                        r = wi % 4
                        q = wi // 4
                        if wi < 16:
                            c, b, M, K = q, r, TS, KW
                        else:
                            j = wi - 16
                            c, b = (j + img) % 4, 4
                            M, K = (TS, KW) if j < 3 else (TAILM, TAILK)
                        nc.tensor.matmul(
                            ps[b][32 * c:32 * c + M, :],
                            toep_sb[32 * r:32 * r + K,
                                    TS * v:TS * v + M],
                            xt[32 * r:32 * r + K,
                               HP * q + v:HP * q + v + 512],
                            start=(v == 0),
                            stop=(v == KS - 1),
                            tile_position=(32 * r, 32 * c),
                        )

                tile_a = oa_pool.tile([128, 2048], f16, name="oa",
                                      tag="oa")
                if img % 2 == 0:
                    tile_b = ob_pool.tile([128, 1024], f16, name="ob",
                                          tag="ob")

                # full-bank evacuations (all 128 partition lanes)
                for k in range(4):
                    if ec % 2 == 0:
                        nc.vector.tensor_copy(
                            tile_a[:, 512 * k:512 * k + 512], ps[k][:])
                    else:
                        nc.scalar.copy(
                            tile_a[:, 512 * k:512 * k + 512], ps[k][:])
                    ec += 1
                # tail bank: un-permute c-groups back to j order (26-lane
                # copies, but only 4 per image)
                for j in range(4):
                    c = (j + img) % 4
                    M = TS if j < 3 else TAILM
                    dstb = tile_b[32 * j:32 * j + M,
                                  512 * (img % 2):512 * (img % 2) + 512]
                    if ec % 2 == 0:
                        nc.vector.tensor_copy(dstb,
                                              ps[4][32 * c:32 * c + M, :])
                    else:
                        nc.scalar.copy(dstb, ps[4][32 * c:32 * c + M, :])
                    ec += 1

                # dense-pack in the DMA: DRAM rows 26c+m <- SBUF parts 32c+m
                for c in range(4):
                    nc.scalar.dma_start(
                        out=oa_ext[img, 26 * c:26 * c + TS, :],
                        in_=tile_a[32 * c:32 * c + TS, :])
                if img % 2 == 1:
                    for j in range(4):
                        M = TS if j < 3 else TAILM
                        nc.gpsimd.dma_start(
                            out=ob_ext[img // 2, 26 * j:26 * j + M, :],
                            in_=tile_b[32 * j:32 * j + M, :])
    nc.finalize()
    return nc


def _host_prep(x, w):
    x = np.asarray(x, dtype=np.float32)
    w = np.asarray(w, dtype=np.float32)
    xpad = np.zeros((B, XROWS, HP), dtype=np.float16)
    xpad[:, PAD:PAD + H, PAD:PAD + W] = x
    p = np.arange(128)
    q = np.arange(NSLAB)
    ridx = (104 * q[None, :] + 26 * (p[:, None] // 32)
            + np.minimum(p[:, None] % 32, 25))
    xslab = np.ascontiguousarray(
        xpad[:, ridx, :].reshape(B, 128, NSLAB * HP))
    toep = np.zeros((128, KS * TS), dtype=np.float16)
    w16 = w.astype(np.float16)
    m = np.arange(TS)
    for st in range(4):
        for v in range(KS):
            for d in range(KS):
                toep[32 * st + m + d, TS * v + m] = w16[d, v]
    return xslab, toep


def _execute(x, w, **run_kwargs):
    from concourse.bass_utils import run_bass_kernel_spmd

    xslab, toep = _host_prep(x, w)
    nc = _build_program()
    in_maps = [
        {"x": xslab[c * PER_CORE:(c + 1) * PER_CORE], "toep": toep}
        for c in range(N_CORES)
    ]
    last_err = None
    for _attempt in range(3):
        try:
            res = run_bass_kernel_spmd(nc, in_maps,
                                       core_ids=list(range(N_CORES)),
                                       **run_kwargs)
            break
        except Exception as e:  # transient NRT execute flakes -> retry
            last_err = e
    else:
        raise last_err
    out = np.empty((B, H, W), dtype=np.float32)
    for c in range(N_CORES):
        sl = slice(c * PER_CORE, (c + 1) * PER_CORE)
        oa = np.asarray(res.results[c]["oa"], dtype=np.float32)
        ob = np.asarray(res.results[c]["ob"], dtype=np.float32)
        oa6 = oa.reshape(PER_CORE, 4, 26, 4, 512)
        out[sl, :416, :] = oa6.transpose(0, 1, 3, 2, 4).reshape(
            PER_CORE, 416, 512)
        ob4 = ob.reshape(PER_CORE // 2, 96, 2, 512)
        out[sl, 416:, :] = ob4.transpose(0, 2, 1, 3).reshape(
            PER_CORE, 96, 512)
    return out, res


def kernel(x, w):
    out, _ = _execute(x, w)
    return out


# revision 8
# speedup vs baseline: 3.6824x; 1.3162x over previous
"""7x7 'same' 2D convolution over [128, 512, 512] f32, data-parallel on 8 NeuronCores.

Banded-Toeplitz formulation with 32x32 TensorEngine array packing: the PE
array runs as 16 independent 32x32 tiles. Each output-row window w (26 rows,
stride 26; window 19 is an 18-row tail) maps to tile position
(32*(w%4), 32*c): moving data = 32 input rows (26 loaded from HBM + 6-row
halo built by SBUF->SBUF DMA), stationary = the [32, 26] Toeplitz band
T_v[r', m] = w[r'-m, v]. The 7 column taps (v) accumulate into PSUM via
shifted moving views. 20 windows/image over 5 PSUM banks; the tail bank's
column group rotates with the image index to balance tile occupancy.

Inputs are cast fp16 and pre-staged host-side into a 5-slab layout
(partition 32r+p', slab q, col) = padded row 104q + 26r + p' for p' < 26
(halo partitions 26..31 of each strip are never transferred). PSUM banks are
evacuated (VectorE/ScalarE alternating) into densely packed fp16 tiles
([104, 2048] main + [96, 1024] tail-pair) so stores move exactly the output
bytes with 2-4KB DMA descriptors; the host un-permutes. Loads + halo copies
own the sync HWDGE ring; stores alternate scalar HWDGE / gpsimd SWDGE.
"""

import numpy as np

B, H, W = 128, 512, 512
KS = 7
PAD = (KS - 1) // 2          # 3
HP = H + 2 * PAD             # 518
N_CORES = 8
PER_CORE = B // N_CORES      # 16
TS = 26                      # output rows per 32x32 window
KW = 32                      # input rows per window
NW = 20                      # windows per image
TAILM = H - 19 * TS          # 18
TAILK = TAILM + KS - 1       # 24
NSLAB = 5
XROWS = 528                  # padded rows addressed by the slab layout


def _build_program():
    import concourse.bass as bass
    import concourse.tile as tile
    from concourse import bacc, mybir

    f16 = mybir.dt.float16
    f32 = mybir.dt.float32

    nc = bacc.Bacc("TRN2", target_bir_lowering=False, debug=False,
                   num_devices=N_CORES)
    x_ext = nc.declare_dram_parameter("x", [PER_CORE, 128, NSLAB * HP], f16,
                                      isOutput=False)
    t_ext = nc.declare_dram_parameter("toep", [128, KS * TS], f16,
                                      isOutput=False)
    # oa[img] packs windows 0..15: row 26c+m, col 512r+j = out row
    # 104c + 26r + m, col j
    oa_ext = nc.declare_dram_parameter("oa", [PER_CORE, 104, 2048], f16,
                                       isOutput=True)
    # ob[ip] packs the tails of images 2ip, 2ip+1: row 26j+m,
    # col 512*par + j' = img 2ip+par, out row 416 + 26j + m
    ob_ext = nc.declare_dram_parameter("ob", [PER_CORE // 2, 96, 1024], f16,
                                       isOutput=True)

    with tile.TileContext(nc) as tc:
        with (
            tc.tile_pool(name="toep", bufs=1) as toep_pool,
            tc.tile_pool(name="x", bufs=3) as x_pool,
            tc.tile_pool(name="oa", bufs=3) as oa_pool,
            tc.tile_pool(name="ob", bufs=2) as ob_pool,
            tc.tile_pool(name="psum", bufs=8, space="PSUM") as psum_pool,
        ):
            toep_sb = toep_pool.tile([128, KS * TS], f16)
            nc.sync.dma_start(out=toep_sb[:], in_=t_ext[:])

            xts = {}

            def load_img(i):
                xt = x_pool.tile([128, NSLAB * HP], f16, name="xt",
                                 tag="xt")
                nc.sync.dma_start(out=xt[:], in_=x_ext[i])
                xts[i] = xt

            ec = 0          # evac round-robin counter
            tile_b = None
            load_img(0)
            load_img(1)
            for img in range(PER_CORE):
                if img + 2 < PER_CORE:
                    load_img(img + 2)
                xt = xts.pop(img)

                ps = [psum_pool.tile([128, 512], f32, name=f"ps{k}",
                                     tag="acc") for k in range(5)]
                # v-major emission: consecutive matmuls hit different tile
                # positions so all 16 PE tiles stream concurrently
                for v in range(KS):
                    for wi in range(NW):
                        r = wi % 4
                        q = wi // 4
                        if wi < 16:
                            c, b, M, K = q, r, TS, KW
                        else:
                            j = wi - 16
                            c, b = (j + img) % 4, 4
                            M, K = (TS, KW) if j < 3 else (TAILM, TAILK)
                        nc.tensor.matmul(
                            ps[b][32 * c:32 * c + M, :],
                            toep_sb[32 * r:32 * r + K,
                                    TS * v:TS * v + M],
                            xt[32 * r:32 * r + K,
                               HP * q + v:HP * q + v + 512],
                            start=(v == 0),
                            stop=(v == KS - 1),
                            tile_position=(32 * r, 32 * c),
                        )

                tile_a = oa_pool.tile([128, 2048], f16, name="oa",
                                      tag="oa")
                if img % 2 == 0:
                    tile_b = ob_pool.tile([128, 1024], f16, name="ob",
                                          tag="ob")

                # full-bank evacuations (all 128 partition lanes)
                for k in range(4):
                    if ec % 2 == 0:
                        nc.vector.tensor_copy(
                            tile_a[:, 512 * k:512 * k + 512], ps[k][:])
                    else:
                        nc.scalar.copy(
                            tile_a[:, 512 * k:512 * k + 512], ps[k][:])
                    ec += 1
                # tail bank: un-permute c-groups back to j order (26-lane
                # copies, but only 4 per image)
                for j in range(4):
                    c = (j + img) % 4
                    M = TS if j < 3 else TAILM
                    dstb = tile_b[32 * j:32 * j + M,
                                  512 * (img % 2):512 * (img % 2) + 512]
                    if ec % 2 == 0:
                        nc.vector.tensor_copy(dstb,
                                              ps[4][32 * c:32 * c + M, :])
                    else:
                        nc.scalar.copy(dstb, ps[4][32 * c:32 * c + M, :])
                    ec += 1

                # dense-pack in the DMA: DRAM rows 26c+m <- SBUF parts 32c+m
                for c in range(4):
                    nc.scalar.dma_start(
                        out=oa_ext[img, 26 * c:26 * c + TS, :],
                        in_=tile_a[32 * c:32 * c + TS, :])
                if img % 2 == 1:
                    for j in range(4):
                        M = TS if j < 3 else TAILM
                        nc.gpsimd.dma_start(
                            out=ob_ext[img // 2, 26 * j:26 * j + M, :],
                            in_=tile_b[32 * j:32 * j + M, :])
    nc.finalize()
    return nc


def _host_prep(x, w):
    x = np.asarray(x, dtype=np.float32)
    w = np.asarray(w, dtype=np.float32)
    xpad = np.zeros((B, XROWS, HP), dtype=np.float16)
    xpad[:, PAD:PAD + H, PAD:PAD + W] = x
    p = np.arange(128)
    q = np.arange(NSLAB)
    # halo partitions (p%32 >= 26) hold the next window's first rows
    ridx = 104 * q[None, :] + 26 * (p[:, None] // 32) + p[:, None] % 32
    xslab = np.ascontiguousarray(
        xpad[:, ridx, :].reshape(B, 128, NSLAB * HP))
    toep = np.zeros((128, KS * TS), dtype=np.float16)
    w16 = w.astype(np.float16)
    m = np.arange(TS)
    for st in range(4):
        for v in range(KS):
            for d in range(KS):
                toep[32 * st + m + d, TS * v + m] = w16[d, v]
    return xslab, toep


def _execute(x, w, **run_kwargs):
    from concourse.bass_utils import run_bass_kernel_spmd

    xslab, toep = _host_prep(x, w)
    nc = _build_program()
    in_maps = [
        {"x": xslab[c * PER_CORE:(c + 1) * PER_CORE], "toep": toep}
        for c in range(N_CORES)
    ]
    last_err = None
    for _attempt in range(3):
        try:
            res = run_bass_kernel_spmd(nc, in_maps,
                                       core_ids=list(range(N_CORES)),
                                       **run_kwargs)
            break
        except Exception as e:  # transient NRT execute flakes -> retry
            last_err = e
    else:
        raise last_err
    out = np.empty((B, H, W), dtype=np.float32)
    for c in range(N_CORES):
        sl = slice(c * PER_CORE, (c + 1) * PER_CORE)
        oa = np.asarray(res.results[c]["oa"], dtype=np.float32)
        ob = np.asarray(res.results[c]["ob"], dtype=np.float32)
        oa6 = oa.reshape(PER_CORE, 4, 26, 4, 512)
        out[sl, :416, :] = oa6.transpose(0, 1, 3, 2, 4).reshape(
            PER_CORE, 416, 512)
        ob4 = ob.reshape(PER_CORE // 2, 96, 2, 512)
        out[sl, 416:, :] = ob4.transpose(0, 2, 1, 3).reshape(
            PER_CORE, 96, 512)
    return out, res


def kernel(x, w):
    out, _ = _execute(x, w)
    return out


# revision 9
# speedup vs baseline: 3.7058x; 1.0064x over previous
"""7x7 'same' 2D convolution over [128, 512, 512] f32, data-parallel on 8 NeuronCores.

Banded-Toeplitz formulation with 32x32 TensorEngine array packing: the PE
array runs as 16 independent 32x32 tiles. Each output-row window w (26 rows,
stride 26; window 19 is an 18-row tail) maps to tile position
(32*(w%4), 32*c): moving data = 32 input rows (26 loaded from HBM + 6-row
halo built by SBUF->SBUF DMA), stationary = the [32, 26] Toeplitz band
T_v[r', m] = w[r'-m, v]. The 7 column taps (v) accumulate into PSUM via
shifted moving views. 20 windows/image over 5 PSUM banks; the tail bank's
column group rotates with the image index to balance tile occupancy.

Inputs are cast fp16 and pre-staged host-side into a 5-slab layout
(partition 32r+p', slab q, col) = padded row 104q + 26r + p' for p' < 26
(halo partitions 26..31 of each strip are never transferred). PSUM banks are
evacuated (VectorE/ScalarE alternating) into densely packed fp16 tiles
([104, 2048] main + [96, 1024] tail-pair) so stores move exactly the output
bytes with 2-4KB DMA descriptors; the host un-permutes. Loads + halo copies
own the sync HWDGE ring; stores alternate scalar HWDGE / gpsimd SWDGE.
"""

import numpy as np

B, H, W = 128, 512, 512
KS = 7
PAD = (KS - 1) // 2          # 3
HP = H + 2 * PAD             # 518
N_CORES = 8
PER_CORE = B // N_CORES      # 16
TS = 26                      # output rows per 32x32 window
KW = 32                      # input rows per window
NW = 20                      # windows per image
TAILM = H - 19 * TS          # 18
TAILK = TAILM + KS - 1       # 24
NSLAB = 5
XROWS = 528                  # padded rows addressed by the slab layout


def _build_program():
    import concourse.bass as bass
    import concourse.tile as tile
    from concourse import bacc, mybir

    f16 = mybir.dt.float16
    bf16 = mybir.dt.bfloat16
    f32 = mybir.dt.float32

    nc = bacc.Bacc("TRN2", target_bir_lowering=False, debug=False,
                   num_devices=N_CORES)
    x_ext = nc.declare_dram_parameter("x", [PER_CORE, 128, NSLAB * HP], bf16,
                                      isOutput=False)
    t_ext = nc.declare_dram_parameter("toep", [128, KS * TS], bf16,
                                      isOutput=False)
    # oa[img] packs windows 0..15: row 26c+m, col 512r+j = out row
    # 104c + 26r + m, col j
    oa_ext = nc.declare_dram_parameter("oa", [PER_CORE, 104, 2048], f16,
                                       isOutput=True)
    # ob[ip] packs the tails of images 2ip, 2ip+1: row 26j+m,
    # col 512*par + j' = img 2ip+par, out row 416 + 26j + m
    ob_ext = nc.declare_dram_parameter("ob", [PER_CORE // 2, 96, 1024], f16,
                                       isOutput=True)

    with tile.TileContext(nc) as tc:
        with (
            tc.tile_pool(name="toep", bufs=1) as toep_pool,
            tc.tile_pool(name="x", bufs=3) as x_pool,
            tc.tile_pool(name="oa", bufs=3) as oa_pool,
            tc.tile_pool(name="ob", bufs=2) as ob_pool,
            tc.tile_pool(name="psum", bufs=8, space="PSUM") as psum_pool,
        ):
            toep_sb = toep_pool.tile([128, KS * TS], bf16)
            nc.sync.dma_start(out=toep_sb[:], in_=t_ext[:])

            xts = {}

            def load_img(i):
                xt = x_pool.tile([128, NSLAB * HP], bf16, name="xt",
                                 tag="xt")
                nc.sync.dma_start(out=xt[:], in_=x_ext[i])
                xts[i] = xt

            ec = 0          # evac round-robin counter
            tile_b = None
            load_img(0)
            load_img(1)
            for img in range(PER_CORE):
                if img + 2 < PER_CORE:
                    load_img(img + 2)
                xt = xts.pop(img)

                ps = [psum_pool.tile([128, 512], f32, name=f"ps{k}",
                                     tag="acc") for k in range(5)]
                # v-major emission: consecutive matmuls hit different tile
                # positions so all 16 PE tiles stream concurrently
                for v in range(KS):
                    for wi in range(NW):
                        r = wi % 4
                        q = wi // 4
                        if wi < 16:
                            c, b, M, K = q, r, TS, KW
                        else:
                            j = wi - 16
                            c, b = (j + img) % 4, 4
                            M, K = (TS, KW) if j < 3 else (TAILM, TAILK)
                        nc.tensor.matmul(
                            ps[b][32 * c:32 * c + M, :],
                            toep_sb[32 * r:32 * r + K,
                                    TS * v:TS * v + M],
                            xt[32 * r:32 * r + K,
                               HP * q + v:HP * q + v + 512],
                            start=(v == 0),
                            stop=(v == KS - 1),
                            tile_position=(32 * r, 32 * c),
                        )

                tile_a = oa_pool.tile([128, 2048], f16, name="oa",
                                      tag="oa")
                if img % 2 == 0:
                    tile_b = ob_pool.tile([128, 1024], f16, name="ob",
                                          tag="ob")

                # full-bank evacuations (all 128 partition lanes)
                for k in range(4):
                    if ec % 2 == 0:
                        nc.vector.tensor_copy(
                            tile_a[:, 512 * k:512 * k + 512], ps[k][:])
                    else:
                        nc.scalar.copy(
                            tile_a[:, 512 * k:512 * k + 512], ps[k][:])
                    ec += 1
                # tail bank: un-permute c-groups back to j order (26-lane
                # copies, but only 4 per image)
                for j in range(4):
                    c = (j + img) % 4
                    M = TS if j < 3 else TAILM
                    dstb = tile_b[32 * j:32 * j + M,
                                  512 * (img % 2):512 * (img % 2) + 512]
                    if ec % 2 == 0:
                        nc.vector.tensor_copy(dstb,
                                              ps[4][32 * c:32 * c + M, :])
                    else:
                        nc.scalar.copy(dstb, ps[4][32 * c:32 * c + M, :])
                    ec += 1

                # dense-pack in the DMA: DRAM rows 26c+m <- SBUF parts 32c+m
                for c in range(4):
                    nc.scalar.dma_start(
                        out=oa_ext[img, 26 * c:26 * c + TS, :],
                        in_=tile_a[32 * c:32 * c + TS, :])
                if img % 2 == 1:
                    for j in range(4):
                        M = TS if j < 3 else TAILM
                        nc.gpsimd.dma_start(
                            out=ob_ext[img // 2, 26 * j:26 * j + M, :],
                            in_=tile_b[32 * j:32 * j + M, :])
    nc.finalize()
    return nc


def _host_prep(x, w):
    x = np.asarray(x, dtype=np.float32)
    w = np.asarray(w, dtype=np.float32)
    import ml_dtypes
    xpad = np.zeros((B, XROWS, HP), dtype=ml_dtypes.bfloat16)
    xpad[:, PAD:PAD + H, PAD:PAD + W] = x
    p = np.arange(128)
    q = np.arange(NSLAB)
    # halo partitions (p%32 >= 26) hold the next window's first rows
    ridx = 104 * q[None, :] + 26 * (p[:, None] // 32) + p[:, None] % 32
    xslab = np.ascontiguousarray(
        xpad[:, ridx, :].reshape(B, 128, NSLAB * HP))
    toep = np.zeros((128, KS * TS), dtype=ml_dtypes.bfloat16)
    w16 = w.astype(ml_dtypes.bfloat16)
    m = np.arange(TS)
    for st in range(4):
        for v in range(KS):
            for d in range(KS):
                toep[32 * st + m + d, TS * v + m] = w16[d, v]
    return xslab, toep


def _execute(x, w, **run_kwargs):
    from concourse.bass_utils import run_bass_kernel_spmd

    xslab, toep = _host_prep(x, w)
    nc = _build_program()
    in_maps = [
        {"x": xslab[c * PER_CORE:(c + 1) * PER_CORE], "toep": toep}
        for c in range(N_CORES)
    ]
    last_err = None
    for _attempt in range(3):
        try:
            res = run_bass_kernel_spmd(nc, in_maps,
                                       core_ids=list(range(N_CORES)),
                                       **run_kwargs)
            break
        except Exception as e:  # transient NRT execute flakes -> retry
            last_err = e
    else:
        raise last_err
    out = np.empty((B, H, W), dtype=np.float32)
    for c in range(N_CORES):
        sl = slice(c * PER_CORE, (c + 1) * PER_CORE)
        oa = np.asarray(res.results[c]["oa"], dtype=np.float32)
        ob = np.asarray(res.results[c]["ob"], dtype=np.float32)
        oa6 = oa.reshape(PER_CORE, 4, 26, 4, 512)
        out[sl, :416, :] = oa6.transpose(0, 1, 3, 2, 4).reshape(
            PER_CORE, 416, 512)
        ob4 = ob.reshape(PER_CORE // 2, 96, 2, 512)
        out[sl, 416:, :] = ob4.transpose(0, 2, 1, 3).reshape(
            PER_CORE, 96, 512)
    return out, res


def kernel(x, w):
    out, _ = _execute(x, w)
    return out
